# revision 1
# baseline (speedup 1.0000x reference)
"""LDA head (segment-reduce + Mahalanobis scores) on 8 Trainium2 NeuronCores.

Strategy (single SPMD NEFF on 8 cores):
  - Stats are class-sharded: core k owns classes [125k, 125k+125) and scans the
    full batch, computing [S1^T | S2^T | counts] = onehot^T @ [z | z^2 | 1] with
    16 accumulating PE matmuls (one per 128-row batch tile).
  - Per-class mean, log-prior, and the class-reduced pooled-variance partial are
    packed into a (66,128) fp32 block and AllGather'd (33KB/rank, mesh floor).
  - Scores are batch-sharded: core k computes output rows [256k, 256k+256) as
    out = [z^T; 1; 1]^T @ [prec*mean; log prior; -0.5*r]  (+ -0.5*q_b fused into
    the PSUM->SBUF copy as a per-partition activation bias).

kernel(z, y) takes the full inputs and returns the full (2048, 1000) output.
"""

import sys
import numpy as np

if "/opt/trn_rl_repo" not in sys.path:
    sys.path.insert(0, "/opt/trn_rl_repo")

import concourse.bacc as bacc
import concourse.bass as bass
import concourse.mybir as mybir
from concourse import tile
from concourse.bass_utils import run_bass_kernel_spmd

B, C, D = 2048, 1000, 64
NCORES = 8
CL = C // NCORES            # 125 classes per core
NT = B // 128               # 16 batch tiles
BL = B // NCORES            # 256 output rows per core
JT = BL // 128              # 2 local batch tiles
EPS_STATS = 1e-5
EPS_PREC = 1e-6
TSUM = float(np.float32(B) + np.float32(C * EPS_STATS))   # counts.sum()
FP = mybir.dt.float32
AF = mybir.ActivationFunctionType
ALU = mybir.AluOpType


def build_program(use_collective=True):
    nc = bacc.Bacc("TRN2", target_bir_lowering=False, debug=False,
                   num_devices=NCORES)

    z_in = nc.dram_tensor("z_in", [B, D], FP, kind="ExternalInput")
    ycols = nc.dram_tensor("ycols", [128, NT], FP, kind="ExternalInput")
    cvals = nc.dram_tensor("cvals", [128, CL], FP, kind="ExternalInput")
    zloc = nc.dram_tensor("zloc", [128, JT, D], FP, kind="ExternalInput")
    ident = nc.dram_tensor("ident", [128, 128], FP, kind="ExternalInput")
    out = nc.dram_tensor("out_loc", [BL, C], FP, kind="ExternalOutput")

    with tile.TileContext(nc) as tc:
        with tc.tile_pool(name="sb", bufs=1) as pool, \
             tc.tile_pool(name="ps", bufs=8, space="PSUM") as pp, \
             tc.tile_pool(name="dram", bufs=1, space="DRAM") as dr:

            g_in = dr.tile([66, 128], FP)
            if use_collective:
                g_out = dr.tile([NCORES, 66, 128], FP, addr_space="Shared")
            else:
                g_out = dr.tile([NCORES, 66, 128], FP)

            # ---- input DMAs -------------------------------------------------
            cv = pool.tile([128, CL], FP)
            nc.sync.dma_start(cv[:], cvals[:, :])
            yc = pool.tile([128, NT], FP)
            nc.sync.dma_start(yc[:], ycols[:, :])
            idn = pool.tile([128, 128], FP)
            nc.sync.dma_start(idn[:], ident[:, :])
            zl = pool.tile([128, JT, D], FP)
            nc.sync.dma_start(zl[:], zloc[:, :, :])
            M = pool.tile([128, NT, 130], FP)      # [z | z^2 | 1 | pad]
            nc.sync.dma_start(M[:, :, 0:D],
                              z_in[:, :].rearrange("(t p) d -> p t d", p=128))

            # ---- phase A: class-sharded segment stats ----------------------
            nc.scalar.activation(M[:, :, D:2 * D], M[:, :, 0:D], AF.Square)
            nc.vector.memset(M[:, :, 2 * D:2 * D + 1], 1.0)

            # all on DVE: gpsimd shares SBUF ports with DVE and slows
            # these ops ~7x when run concurrently
            oh = pool.tile([128, NT, CL], FP)
            for t in range(NT):
                nc.vector.tensor_scalar(oh[:, t, :], cv[:], yc[:, t:t + 1],
                                        None, ALU.is_equal)

            psS = pp.tile([CL, 129], FP, tag="ps")
            for t in range(NT):
                nc.tensor.matmul(psS[:], lhsT=oh[:, t, :], rhs=M[:, t, 0:129],
                                 start=(t == 0), stop=(t == NT - 1))

            # stats post-processing, all in class-partition layout
            cnt = pool.tile([CL, 1], FP)
            nc.vector.tensor_scalar_add(cnt[:], psS[:, 128:129], EPS_STATS)
            rcp = pool.tile([CL, 1], FP)
            nc.vector.reciprocal(rcp[:], cnt[:])

            TB = pool.tile([CL, 66], FP)           # [mean^T | beta | p-col]
            nc.vector.memset(TB[:, 65:66], 0.0)
            nc.vector.tensor_scalar(TB[:, 0:64], psS[:, 0:64], rcp[:], None,
                                    ALU.mult)
            nc.scalar.activation(TB[:, 64:65], cnt[:], AF.Ln,
                                 scale=1.0 / TSUM)

            cnt2 = pool.tile([CL, 1], FP)
            nc.vector.tensor_scalar_add(cnt2[:], psS[:, 128:129],
                                        2.0 * EPS_STATS)
            rcp2 = pool.tile([CL, 1], FP)
            nc.vector.tensor_tensor(rcp2[:], rcp[:], rcp[:], ALU.mult)
            alph = pool.tile([CL, 1], FP)
            nc.vector.tensor_tensor(alph[:], cnt2[:], rcp2[:], ALU.mult)

            s1sq = pool.tile([CL, 64], FP)
            nc.scalar.activation(s1sq[:], psS[:, 0:64], AF.Square)
            t1 = pool.tile([CL, 64], FP)
            nc.vector.tensor_scalar(t1[:], s1sq[:], alph[:], None, ALU.mult)
            ptile = pool.tile([CL, 64], FP)
            nc.vector.tensor_tensor(ptile[:], psS[:, 64:128], t1[:],
                                    ALU.subtract)

            onesc = pool.tile([CL, 1], FP)
            nc.vector.memset(onesc[:], 1.0)
            psP = pp.tile([64, 1], FP, tag="ps")
            nc.tensor.matmul(psP[:], lhsT=ptile[:], rhs=onesc[:],
                             start=True, stop=True)
            nc.scalar.copy(TB[0:64, 65:66], psP[:])

            psT = pp.tile([66, 125], FP, tag="ps")
            nc.tensor.transpose(psT[:], TB[:, :], idn[0:CL, 0:CL])
            G = pool.tile([66, 128], FP)
            nc.vector.memset(G[:, 125:128], 0.0)
            nc.scalar.copy(G[:, 0:125], psT[:])
            nc.sync.dma_start(g_in[:], G[:, :])

            # local z^T for the score matmuls (overlaps the collective)
            zTq = pool.tile([66, 256], FP)
            nc.vector.memset(zTq[64:66, :], 1.0)
            for j in range(JT):
                psZ = pp.tile([64, 128], FP, tag="ps")
                nc.tensor.transpose(psZ[:], zl[:, j, :], idn[:, :])
                nc.scalar.copy(zTq[0:64, j * 128:(j + 1) * 128], psZ[:])
            zsq = pool.tile([128, JT, D], FP)
            nc.scalar.activation(zsq[:], zl[:], AF.Square)

            # ---- AllGather of (mean, beta, pooled-partial) ------------------
            if use_collective:
                nc.gpsimd.collective_compute(
                    "AllGather", ALU.bypass,
                    replica_groups=[list(range(NCORES))],
                    ins=[g_in.opt()], outs=[g_out.opt()],
                )
            else:
                # debug: replicate local block into every rank slot
                for k in range(NCORES):
                    nc.sync.dma_start(g_out[k, :, :], g_in[:, :])

            # ---- phase B: batch-sharded Mahalanobis scores ------------------
            msb = pool.tile([64, C], FP)
            nc.sync.dma_start(msb[:].rearrange("d (k c) -> d k c", k=NCORES),
                              g_out[:, 0:64, 0:CL].rearrange("k d c -> d k c"))
            V = pool.tile([66, C], FP)              # [prec*mean; -r/2; beta]
            nc.sync.dma_start(
                V[65:66, :].rearrange("o (k c) -> o k c", k=NCORES),
                g_out[:, 64:65, 0:CL].rearrange("k o c -> o k c"))
            ppart = pool.tile([64, NCORES], FP)
            nc.sync.dma_start(
                ppart[:],
                g_out[:, 65:66, 0:64].rearrange("k o d -> d (o k)"))

            ptot = pool.tile([64, 1], FP)
            nc.vector.reduce_sum(ptot[:], ppart[:], axis=mybir.AxisListType.X)
            pooled = pool.tile([64, 1], FP)
            nc.vector.tensor_scalar(pooled[:], ptot[:], 1.0 / TSUM,
                                    EPS_STATS, ALU.mult, ALU.add)
            pmax = pool.tile([64, 1], FP)
            nc.vector.tensor_scalar_max(pmax[:], pooled[:], EPS_PREC)
            prec = pool.tile([64, 1], FP)
            nc.vector.reciprocal(prec[:], pmax[:])

            # mean^2 is prec-independent: runs concurrently with the
            # reduce/reciprocal chain right after the gather lands
            msq = pool.tile([64, C], FP)
            nc.vector.tensor_tensor(msq[:], msb[:], msb[:], ALU.mult)

            # PE warm-up reads gathered data so it schedules post-gather,
            # right before the r/score matmuls (HAM needs ~3.4us busy)
            junkps = pp.tile([64, 64], FP, tag="ps")
            for w in range(10):
                nc.tensor.matmul(junkps[:], lhsT=idn[0:64, 0:64],
                                 rhs=msb[:, w * 64:w * 64 + 64],
                                 start=True, stop=True)

            nc.vector.tensor_scalar(V[0:64, :], msb[:], prec[:], None,
                                    ALU.mult)
            for h in range(2):
                psR = pp.tile([1, 500], FP, tag="ps")
                nc.tensor.matmul(psR[:], lhsT=prec[:],
                                 rhs=msq[:, h * 500:(h + 1) * 500],
                                 start=True, stop=True)
                nc.scalar.activation(V[64:65, h * 500:(h + 1) * 500], psR[:],
                                     AF.Copy, scale=-0.5)

            # prec broadcast across partitions for the q computation
            psPR = pp.tile([1, 64], FP, tag="ps")
            nc.tensor.transpose(psPR[:], prec[:], idn[0:64, 0:64])
            prow = pool.tile([1, 64], FP)
            nc.vector.tensor_copy(prow[:], psPR[:])
            onesr = pool.tile([1, 128], FP)
            nc.vector.memset(onesr[:], 1.0)
            psPB = pp.tile([128, 64], FP, tag="ps")
            nc.tensor.matmul(psPB[:], lhsT=onesr[:], rhs=prow[:],
                             start=True, stop=True)
            precbc = pool.tile([128, 64], FP)
            nc.vector.tensor_copy(precbc[:], psPB[:])

            junk = pool.tile([128, 64], FP)
            qraw = pool.tile([128, JT], FP)
            qsb = pool.tile([128, JT], FP)
            for j in range(JT):
                nc.vector.tensor_tensor(junk[:], zsq[:, j, :], precbc[:],
                                        ALU.mult)
                nc.vector.reduce_sum(qraw[:, j:j + 1], junk[:],
                                     axis=mybir.AxisListType.X)
            nc.vector.tensor_scalar_mul(qsb[:], qraw[:], -0.5)

            for j in range(JT):
                outj = pool.tile([128, C], FP, tag=f"outsb{j}")
                for h in range(2):
                    psO = pp.tile([128, 500], FP, tag="ps")
                    nc.tensor.matmul(psO[:],
                                     lhsT=zTq[:, j * 128:(j + 1) * 128],
                                     rhs=V[:, h * 500:(h + 1) * 500],
                                     start=True, stop=True)
                    nc.scalar.activation(outj[:, h * 500:(h + 1) * 500],
                                         psO[:], AF.Identity,
                                         bias=qsb[:, j:j + 1], scale=1.0)
                nc.sync.dma_start(out[j * 128:(j + 1) * 128, :], outj[:])

    nc.compile()
    return nc


_NC_CACHE = None


def _get_program():
    global _NC_CACHE
    if _NC_CACHE is None:
        _NC_CACHE = build_program()
    return _NC_CACHE


def make_in_maps(z, y):
    z = np.ascontiguousarray(np.asarray(z, dtype=np.float32))
    yf = np.asarray(y).astype(np.float32)          # labels < 1000, exact
    ycols_np = np.ascontiguousarray(yf.reshape(NT, 128).T)
    ident_np = np.eye(128, dtype=np.float32)
    in_maps = []
    for k in range(NCORES):
        cvals_np = np.broadcast_to(
            np.arange(k * CL, (k + 1) * CL, dtype=np.float32), (128, CL))
        zloc_np = np.ascontiguousarray(
            z[k * BL:(k + 1) * BL].reshape(JT, 128, D).transpose(1, 0, 2))
        in_maps.append({
            "z_in": z,
            "ycols": ycols_np,
            "cvals": np.ascontiguousarray(cvals_np),
            "zloc": zloc_np,
            "ident": ident_np,
        })
    return in_maps


def run(z, y, trace=False, **kwargs):
    nc = _get_program()
    res = run_bass_kernel_spmd(nc, make_in_maps(z, y), list(range(NCORES)),
                               trace=trace, **kwargs)
    full = np.concatenate([res.results[k]["out_loc"] for k in range(NCORES)],
                          axis=0)
    return full, res


def kernel(z, y):
    full, _ = run(z, y, trace=False)
    return full


if __name__ == "__main__":
    rng = np.random.default_rng(0)
    z = rng.standard_normal((B, D), dtype=np.float32)
    y = rng.integers(0, C, size=(B,)).astype(np.int64)
    out = kernel(z, y)
    print("out", out.shape, out.dtype, out[0, :4])

